# revision 29
# baseline (speedup 1.0000x reference)
"""Trainium2 Bass kernel for nn_AttentionBlock (GroupNorm + single-head spatial
self-attention + residual) on 8 NeuronCores.

Sharding: data-parallel over batch (2) x sequence-parallel over the query
dimension (4 chunks of 1024 of the 4096 spatial tokens). Each core gets the
full image of its batch element, ROTATED so its query chunk sits at token 0
(GroupNorm stats, key/value sets and softmax sums are permutation-invariant
over tokens, so rotation lets all 8 cores run the identical SPMD program).

v2: fp8(e4m3) DoubleRow matmuls everywhere except the final wp projection.
  - x streams to SBUF ONCE (f32r resident); GroupNorm stats (bn_stats) and
    warm-up matmuls (keeps the PE HAM un-throttled through the DMA-bound
    stats phase) consume it chunk-wise as it lands.
  - the GroupNorm scale a folds into the fp8 x copy (xn8 = a*x, quantize);
    the shift b becomes per-output-channel constants via tiny fp8 matmuls:
    q keeps qb, k's bias is DROPPED (a per-query additive logit constant is
    softmax-invariant), v's bias rides through softmax into yb.
  - projections: weights pre-quantized to fp8 on the host in DoubleRow pair
    layout [pair][128, 2, 512]; k/q use weight-stationary DoubleRow matmuls
    (contraction 256/step at 0.5 cyc/row), v uses x-stationary to produce
    vT [token, channel] directly (no transposes anywhere).
  - attention: scores^T per 128-key tile = 2 DoubleRow MMs; exp on ScalarE
    (shifted by -2.0 to keep exp < 448 = fp8 max) straight to fp8 pairs;
    row-sums via a DoubleRow ones-matmul; AV accumulates over 16 key-pairs.
  - softmax normalization deferred: 1/r via exp(-ln r) on ScalarE, broadcast
    with a K=1 matmul, folded into the PSUM->SBUF move; y = wp @ hattn (f32r)
    + yb + x (from the resident x) in one fused DVE op per tile.
"""

import sys
from contextlib import ExitStack

if "/opt/trn_rl_repo" not in sys.path:
    sys.path.insert(0, "/opt/trn_rl_repo")

import ml_dtypes
import numpy as np

import concourse.bass as bass  # noqa: F401  (import keeps bass registered)
import concourse.tile as tile
from concourse import bacc, mybir
from concourse.alu_op_type import AluOpType
from concourse.bass_utils import run_bass_kernel_spmd

F32 = mybir.dt.float32
F32R = mybir.dt.float32r
F8 = mybir.dt.float8e4
AF = mybir.ActivationFunctionType
OP = AluOpType
DR = mybir.MatmulPerfMode.DoubleRow

B, C, H, W = 2, 512, 64, 64
HW = H * W          # 4096 spatial tokens
P = 128             # partitions
CT = C // P         # 4 channel tiles
CP = CT // 2        # 2 channel-tile pairs (DoubleRow contraction groups)
NCORES = 8
QN = HW // 4        # 1024 queries per core
CHW = 512           # token chunk width
NCH = HW // CHW     # 8 chunks
JT = HW // P        # 32 key tiles
JP = JT // 2        # 16 key-tile pairs
EPS = 1e-6
SCALE = float(C) ** -0.5
SHIFT = 2.0         # exp(logit - SHIFT): keeps max exp ~190 < 448 (fp8 max)
GPT = P // 16       # 8 groups per channel tile


def _build_body(nc, tc, ctx, d):
    xb_d = d["xb"]
    y_d = d["y"]

    cpool = ctx.enter_context(tc.tile_pool(name="const", bufs=1))
    ppool = ctx.enter_context(tc.tile_pool(name="persist", bufs=1))
    spool = ctx.enter_context(tc.tile_pool(name="stream", bufs=2))
    smpool = ctx.enter_context(tc.tile_pool(name="small", bufs=1))
    qpool = ctx.enter_context(tc.tile_pool(name="psum", bufs=2, space="PSUM"))

    dma_engines = [nc.gpsimd, nc.scalar, nc.sync]

    # ---- phase 1: stream x to SBUF once; GroupNorm stats per chunk ----
    ind = cpool.tile([P, GPT], F32, tag="ind")
    nc.scalar.dma_start(ind[:], d["ind"][:])
    gps = qpool.tile([GPT, 2 * CT], F32, tag="pa")
    sts = [smpool.tile([P, NCH, 6], F32, tag="st", bufs=CT, name=f"st{t}")
           for t in range(CT)]
    xres = [ppool.tile([P, HW], F32R, tag=f"xr{t}", name=f"xres{t}")
            for t in range(CT)]
    for half in range(NCH // 2):
        for t in range(CT):
            eng = dma_engines[(half * CT + t) % 3]
            eng.dma_start(xres[t][:, half * 2 * CHW:(half + 1) * 2 * CHW],
                          xb_d[half, t])
    # xn8 = fp8 cast of RAW x, built chunk-wise during the DMA-bound stats
    # phase (the GroupNorm scale a folds into the fp8 weights instead)
    xn8 = [ppool.tile([P, 2, HW], F8, tag=f"xn8_{p}", name=f"xn8_{p}")
           for p in range(CP)]
    trash = qpool.tile([P, CHW], F32, tag="trash", bufs=1, name="trash")
    for ch in range(NCH):
        sl = slice(ch * CHW, (ch + 1) * CHW)
        for t in range(CT):
            nc.vector.bn_stats(sts[t][:, ch, :],
                               xres[t][:, ch * CHW:(ch + 1) * CHW])
            if t % 2 == 0:
                nc.scalar.copy(xn8[t // 2][:, t % 2, sl], xres[t][:, sl])
            else:
                nc.vector.tensor_copy(xn8[t // 2][:, t % 2, sl], xres[t][:, sl])
        # dummy matmuls on the freshly-landed chunks keep the PE HAM warm
        # through the DMA-bound stats phase: one ~3.4us burst on chunk 0
        # trips the SHORT window to K=8/8, then one matmul per chunk defeats
        # the MID-idle re-throttle; results are discarded
        for r in range(8 if ch == 0 else 1):
            t = (ch + r) % CT
            nc.tensor.matmul(trash[:], xres[t][:, ch * CHW:ch * CHW + P],
                             xres[t][:, sl], start=True, stop=True)

    # ---- small constants (after the x stream in every trigger queue) ----
    chv = []
    for t in range(CT):
        v = cpool.tile([P, 6], F32, tag=f"chv{t}", name=f"chv{t}")
        nc.gpsimd.dma_start(v[:], d["chv"][t])
        chv.append(v)
    # chv columns: gamma, beta, bq, bk, bv, bp
    indT = cpool.tile([GPT, P], F32, tag="indT")
    nc.gpsimd.dma_start(indT[:], d["indT"][:])
    # f32r projection weights (transposed [c, o]): consumed by the tiny bias
    # contracts and by the one-time a-scaled fp8 quantization below
    wts = {}
    for wi, name in enumerate(("wkT", "wvT", "wqT")):
        wts[name] = []
        for t in range(CT):
            w = cpool.tile([P, C], F32R, tag=f"{name}{t}", name=f"{name}{t}")
            dma_engines[(wi * CT + t) % 3].dma_start(w[:], d[name][t])
            wts[name].append(w)
    # fp8 ones for the DoubleRow row-sum matmul: M=16 columns because the
    # dual-fp8 LDWEIGHTS ISA check requires the pair step to be 16-aligned
    # (and fp8 memset is not a legal ISA instruction -> host constant)
    ones2 = cpool.tile([P, 2, 16], F8, tag="ones2")
    nc.gpsimd.dma_start(ones2[:], d["ones8"][:])
    ones_r32 = smpool.tile([1, P], F32R, tag="onesr32")
    nc.gpsimd.dma_start(ones_r32[:], d["onesr"][:])
    epst = smpool.tile([GPT, 1], F32, tag="eps")
    nc.vector.memset(epst[:], EPS)
    shiftt = smpool.tile([P, 1], F32, tag="shift")
    nc.vector.memset(shiftt[:], -SHIFT)

    for t in range(CT):
        mv = smpool.tile([P, 2], F32, tag="mv", bufs=1)
        nc.vector.bn_aggr(mv[:], sts[t][:])
        sq = smpool.tile([P, 1], F32, tag="sq", bufs=1)
        nc.vector.tensor_tensor(sq[:], mv[:, 0:1], mv[:, 0:1], op=OP.mult)
        s2 = smpool.tile([P, 2], F32, tag="s2", bufs=1)
        nc.vector.tensor_copy(s2[:, 0:1], mv[:, 0:1])
        nc.vector.tensor_tensor(s2[:, 1:2], sq[:], mv[:, 1:2], op=OP.add)
        nc.tensor.matmul(gps[:, 2 * t:2 * t + 2], ind[:], s2[:],
                         start=True, stop=True)

    gst = smpool.tile([GPT, 2 * CT], F32, tag="gst")
    nc.vector.tensor_copy(gst[:], gps[:])
    g3 = gst.rearrange("p (t two) -> p t two", two=2)
    msq = smpool.tile([GPT, CT], F32, tag="msq")
    nc.vector.tensor_tensor(msq[:], g3[:, :, 0], g3[:, :, 0], op=OP.mult)
    varg = smpool.tile([GPT, CT], F32, tag="varg")
    nc.vector.tensor_tensor(varg[:], g3[:, :, 1], msq[:], op=OP.subtract)
    stdg = smpool.tile([GPT, CT], F32, tag="stdg")
    nc.scalar.activation(stdg[:], varg[:], AF.Sqrt, bias=epst[:])
    # interleave (mu_t, rstd_t) columns and broadcast all groups->channels
    # with a single [K=8, M=128, N=8] indicator matmul
    mr = smpool.tile([GPT, 2 * CT], F32, tag="mr")
    mr3 = mr.rearrange("p (t two) -> p t two", two=2)
    nc.vector.tensor_copy(mr3[:, :, 0], g3[:, :, 0])
    nc.vector.reciprocal(mr3[:, :, 1], stdg[:])
    cba = qpool.tile([P, 2 * CT], F32, tag="pa")
    nc.tensor.matmul(cba[:], indT[:], mr[:], start=True, stop=True)
    cb = smpool.tile([P, 2 * CT], F32, tag="cb")
    nc.vector.tensor_copy(cb[:], cba[:])

    # per-channel Scale a (col 0) / Bias b (col 1); bvec = f32r copy of b
    sbts, bvec = [], []
    for t in range(CT):
        sbt = ppool.tile([P, 2], F32, tag=f"sb{t}")
        nc.vector.tensor_tensor(sbt[:, 0:1], cb[:, 2 * t + 1:2 * t + 2],
                                chv[t][:, 0:1], op=OP.mult)
        tmpb = smpool.tile([P, 1], F32, tag="tmpb", bufs=1)
        nc.vector.tensor_tensor(tmpb[:], cb[:, 2 * t:2 * t + 1], sbt[:, 0:1],
                                op=OP.mult)
        nc.vector.tensor_tensor(sbt[:, 1:2], chv[t][:, 1:2], tmpb[:],
                                op=OP.subtract)
        bv_ = ppool.tile([P, 2], F32R, tag=f"bvec{t}", name=f"bvec{t}")
        nc.vector.tensor_copy(bv_[:, 0:1], sbt[:, 1:2])
        nc.vector.tensor_copy(bv_[:, 1:2], sbt[:, 1:2])
        sbts.append(sbt)
        bvec.append(bv_)

    # ---- one-time a-scaled fp8 weight quantization (single rounding) ----
    # w8s[name][p][cp, t, o] = fp8( wT[(2p+t)*128+cp, o] * a[(2p+t)*128+cp] )
    w8s = {}
    for name in ("wkT", "wvT", "wqT"):
        w8s[name] = [cpool.tile([P, 2, C], F8, tag=f"{name}8_{p}",
                                name=f"{name}8_{p}") for p in range(CP)]
        for t in range(CT):
            dst = w8s[name][t // 2][:, t % 2, :]
            if t % 2 == 0:
                nc.scalar.activation(dst, wts[name][t][:], AF.Copy,
                                     scale=sbts[t][:, 0:1])
            else:
                nc.vector.tensor_scalar_mul(dst, wts[name][t][:],
                                            sbts[t][:, 0:1])

    # ---- bias-term constants from UNSCALED weights (tiny N=2 matmuls) ----
    #   qb[o] = sum_c wq[o,c] b[c] + bq    (per-partition add at the q copy)
    #   vbt[c] = sum_cin wv[c,cin] b[cin] + bv   (rides softmax into yb)
    #   yb[o] = sum_c wp[o,c] vbt[c] + bp        (y epilogue constant)
    #   (k needs NO bias: a per-query logit constant is softmax-invariant)
    def bias_contract(wtiles, rhs_tiles, outdt, addcol, tagp, two_col=False):
        outs = []
        for ot in range(CT):
            pb = qpool.tile([P, 2], F32, tag="pa")
            for t in range(CT):
                nc.tensor.matmul(pb[:], wtiles[t][:, ot * P:(ot + 1) * P],
                                 rhs_tiles[t][:, 0:2], start=(t == 0),
                                 stop=(t == CT - 1))
            w = 2 if two_col else 1
            ob = ppool.tile([P, w], outdt, tag=f"{tagp}{ot}", name=f"{tagp}{ot}")
            nc.vector.tensor_scalar(ob[:], pb[:, 0:w],
                                    chv[ot][:, addcol:addcol + 1],
                                    None, OP.add)
            outs.append(ob)
        return outs

    vbt = bias_contract(wts["wvT"], bvec, F32R, 4, "vbt", two_col=True)
    qb = bias_contract(wts["wqT"], bvec, F32, 2, "qb")

    # ---- persistent attention operands (all fp8, DoubleRow layouts) ----
    # k2[p]  : [128, j-tile, pair-slot, 128]   stationary slices [:, j, :, :]
    # q2[p]  : [128, pair-slot, 1024]          moving slices [:, :, i-half]
    # xn8[p] : [128, pair-slot, 4096]          moving (k/q) + stationary (v)
    # vT2[jp]: [128, c-tile, pair-slot, 128]   stationary slices [:, t, :, :]
    k2 = [ppool.tile([P, JT, 2, P], F8, tag=f"k2_{p}", name=f"k2_{p}")
          for p in range(CP)]
    q2 = [ppool.tile([P, 2, QN], F8, tag=f"q2_{p}", name=f"q2_{p}")
          for p in range(CP)]
    vT2 = [ppool.tile([P, CT, 2, P], F8, tag=f"vT2_{jp}", name=f"vT2_{jp}")
           for jp in range(JP)]

    # ---- phase 2: q/k/v fp8 projections, streamed over x chunks ----
    for ch in range(NCH):
        sl = slice(ch * CHW, (ch + 1) * CHW)
        for ot in range(CT):
            pk = qpool.tile([P, CHW], F32, tag="pa")
            for p in range(CP):
                nc.tensor.matmul(pk[:],
                                 w8s["wkT"][p][:, :, ot * P:(ot + 1) * P],
                                 xn8[p][:, :, sl], start=(p == 0),
                                 stop=(p == CP - 1), perf_mode=DR)
            # k write: [128, 4 j-tiles, 1, 128] strided into the pair layout
            nc.vector.tensor_copy(k2[ot // 2][:, 4 * ch:4 * ch + 4, ot % 2, :],
                                  pk[:])
        for tg in range(CT):
            jt = ch * CT + tg
            pv = qpool.tile([P, CHW], F32, tag="pa")
            for p in range(CP):
                nc.tensor.matmul(
                    pv[:], xn8[p][:, :, jt * P:(jt + 1) * P],
                    w8s["wvT"][p][:], start=(p == 0), stop=(p == CP - 1),
                    perf_mode=DR)
            dst = vT2[jt // 2][:, :, jt % 2, :]
            if tg % 2 == 1:
                nc.scalar.copy(dst, pv[:])
            else:
                nc.vector.tensor_copy(dst, pv[:])
            del dst
        if ch * CHW < QN:
            for ot in range(CT):
                pq = qpool.tile([P, CHW], F32, tag="pa")
                for p in range(CP):
                    nc.tensor.matmul(
                        pq[:], w8s["wqT"][p][:, :, ot * P:(ot + 1) * P],
                        xn8[p][:, :, sl], start=(p == 0), stop=(p == CP - 1),
                        perf_mode=DR)
                if ot % 2 == 0:
                    nc.scalar.add(q2[ot // 2][:, ot % 2, sl], pq[:],
                                  qb[ot][:, 0:1])
                else:
                    nc.vector.tensor_scalar(q2[ot // 2][:, ot % 2, sl], pq[:],
                                            qb[ot][:], None, OP.add)

    # ---- phase 3: attention, per query half ----
    # wpT (f32r) loads late: only the y epilogue needs it
    wpT = []
    for t in range(CT):
        w = cpool.tile([P, C], F32R, tag=f"wpT{t}", name=f"wpT{t}")
        nc.sync.dma_start(w[:], d["wpT"][t])
        wpT.append(w)
    yb = []
    for ot in range(CT):
        pb = qpool.tile([P, 2], F32, tag="pa")
        for t in range(CT):
            nc.tensor.matmul(pb[:], wpT[t][:, ot * P:(ot + 1) * P],
                             vbt[t][:, 0:2], start=(t == 0), stop=(t == CT - 1))
        ob = ppool.tile([P, 1], F32, tag=f"yb{ot}", name=f"yb{ot}")
        nc.vector.tensor_scalar(ob[:], pb[:, 0:1], chv[ot][:, 5:6], None, OP.add)
        yb.append(ob)

    def mk_pr():
        return qpool.tile([16, CHW], F32, tag="pr", bufs=1, name="pr")

    def mk_po():
        return [qpool.tile([P, CHW], F32, tag=f"po{t}", name=f"po{t}", bufs=1)
                for t in range(CT)]

    def score_pair(ih, jp):
        """scores^T + exp for key tiles (2jp, 2jp+1) -> one fp8 pT2 pair."""
        isl = slice(ih * CHW, (ih + 1) * CHW)
        pT2 = spool.tile([P, 2, CHW], F8, tag="pT2", bufs=6, name="pT2")
        for jj in range(2):
            j = 2 * jp + jj
            ps_ = qpool.tile([P, CHW], F32, tag="pa", name="ps")
            for p in range(CP):
                nc.tensor.matmul(ps_[:], k2[p][:, j, :, :], q2[p][:, :, isl],
                                 start=(p == 0), stop=(p == CP - 1),
                                 perf_mode=DR)
            nc.scalar.activation(pT2[:, jj, :], ps_[:], AF.Exp,
                                 scale=SCALE, bias=shiftt[:])
        return pT2

    def av_pair(pr, po, jp, pT2):
        nc.tensor.matmul(pr[:], ones2[:], pT2[:], start=(jp == 0),
                         stop=(jp == JP - 1), perf_mode=DR)
        for t in range(CT):
            nc.tensor.matmul(po[t][:], vT2[jp][:, t, :, :], pT2[:],
                             start=(jp == 0), stop=(jp == JP - 1),
                             perf_mode=DR)

    def tail_and_y(pr, po, ih, nsub=1):
        # nsub>1 splits the epilogue into query sub-slices so the final
        # drain pipelines DVE normalize / PE matmul / DMA out
        sw = CHW // nsub
        for sub in range(nsub):
            lo = ih * CHW + sub * sw
            isl = slice(lo, lo + sw)
            psl = slice(sub * sw, (sub + 1) * sw)
            rsb = spool.tile([1, sw], F32R, tag="sx", bufs=3)
            nc.vector.tensor_copy(rsb[:], pr[0:1, psl])
            # 1/r via exp(-ln(r)) on ScalarE, in place: faster than DVE's
            # iterative reciprocal and only one stream-pool slot
            nc.scalar.activation(rsb[:], rsb[:], AF.Ln)
            nc.scalar.activation(rsb[:], rsb[:], AF.Exp, scale=-1.0)
            prb = qpool.tile([P, sw], F32, tag="pa")
            nc.tensor.matmul(prb[:], ones_r32[:], rsb[:], start=True, stop=True)
            rb = spool.tile([P, sw], F32, tag="sx", bufs=3)
            nc.vector.tensor_copy(rb[:], prb[:])
            has = []
            for t in range(CT):
                ha = spool.tile([P, sw], F32R, tag=f"hx{t}", bufs=2)
                nc.vector.tensor_tensor(ha[:], po[t][:, psl], rb[:], op=OP.mult)
                has.append(ha)
            for ot in range(CT):
                py = qpool.tile([P, sw], F32, tag="pa")
                for t in range(CT):
                    nc.tensor.matmul(py[:], wpT[t][:, ot * P:(ot + 1) * P],
                                     has[t][:], start=(t == 0),
                                     stop=(t == CT - 1))
                yt = spool.tile([P, sw], F32, tag="yt", bufs=4, name="yt")
                nc.vector.scalar_tensor_tensor(yt[:], py[:], yb[ot][:, 0:1],
                                               xres[ot][:, isl],
                                               op0=OP.add, op1=OP.add)
                dma_engines[(ot + sub) % 3].dma_start(y_d[ot, :, isl], yt[:])

    KPRE = 4  # half-1 score/exp pairs prefetched into half-0's softmax tail
    pr0 = mk_pr()
    po0 = mk_po()
    for jp in range(JP):
        av_pair(pr0, po0, jp, score_pair(0, jp))
    pr1 = mk_pr()
    pre = [score_pair(1, jp) for jp in range(KPRE)]
    tail_and_y(pr0, po0, 0)
    po1 = mk_po()
    for jp in range(JP):
        pT2 = pre[jp] if jp < KPRE else score_pair(1, jp)
        av_pair(pr1, po1, jp, pT2)
    tail_and_y(pr1, po1, 1, nsub=2)


def build_module():
    nc = bacc.Bacc("TRN2", target_bir_lowering=False, debug=False,
                   num_devices=NCORES)
    d = {
        "xb": nc.dram_tensor("xb", [NCH // 2, CT, P, 2 * CHW], F32R,
                             kind="ExternalInput").ap(),
        "wqT": nc.dram_tensor("wqT", [CT, P, C], F32R,
                              kind="ExternalInput").ap(),
        "wkT": nc.dram_tensor("wkT", [CT, P, C], F32R,
                              kind="ExternalInput").ap(),
        "wvT": nc.dram_tensor("wvT", [CT, P, C], F32R,
                              kind="ExternalInput").ap(),
        "wpT": nc.dram_tensor("wpT", [CT, P, C], F32R,
                              kind="ExternalInput").ap(),
        "chv": nc.dram_tensor("chv", [CT, P, 6], F32, kind="ExternalInput").ap(),
        "ones8": nc.dram_tensor("ones8", [P, 2, 16], F8,
                                kind="ExternalInput").ap(),
        "onesr": nc.dram_tensor("onesr", [1, P], F32R,
                                kind="ExternalInput").ap(),
        "ind": nc.dram_tensor("ind", [P, GPT], F32, kind="ExternalInput").ap(),
        "indT": nc.dram_tensor("indT", [GPT, P], F32, kind="ExternalInput").ap(),
        "y": nc.dram_tensor("y", [CT, P, QN], F32, kind="ExternalOutput").ap(),
    }
    with tile.TileContext(nc) as tc, ExitStack() as ctx:
        _build_body(nc, tc, ctx, d)
    nc.compile()
    return nc


_CACHE = {}


def _get_nc():
    if "nc" not in _CACHE:
        _CACHE["nc"] = build_module()
    return _CACHE["nc"]


def _shared_inputs(gamma, beta, wq, bq, wk, bk, wv, bv, wp, bp):
    def wT(w):
        return np.ascontiguousarray(np.asarray(w, np.float32).T).reshape(CT, P, C)

    ind = np.zeros((P, GPT), np.float32)
    for i in range(P):
        ind[i, i // 16] = 1.0 / 16.0
    indT = np.zeros((GPT, P), np.float32)
    for i in range(P):
        indT[i // 16, i] = 1.0
    chv = np.stack([np.asarray(a, np.float32)
                    for a in (gamma, beta, bq, bk, bv, bp)],
                   axis=1).reshape(CT, P, 6)
    return {
        "wqT": wT(wq), "wkT": wT(wk), "wvT": wT(wv),
        "wpT": wT(wp),
        "chv": np.ascontiguousarray(chv),
        "ones8": np.ones((P, 2, 16), np.float32).astype(ml_dtypes.float8_e4m3fn),
        "onesr": np.ones((1, P), np.float32),
        "ind": ind, "indT": indT,
    }


def make_in_maps(x, gamma, beta, wq, bq, wk, bk, wv, bv, wp, bp):
    shared = _shared_inputs(gamma, beta, wq, bq, wk, bk, wv, bv, wp, bp)
    xf = np.asarray(x, np.float32).reshape(B, C, HW)
    in_maps = []
    for core in range(NCORES):
        b, qc = divmod(core, NCORES // B)
        xb = np.roll(xf[b], -qc * QN, axis=1)          # [C, HW]
        xt = xb.reshape(CT, P, NCH // 2, 2 * CHW).transpose(2, 0, 1, 3)
        m = dict(shared)
        m["xb"] = np.ascontiguousarray(xt)
        in_maps.append(m)
    return in_maps


def assemble_output(results):
    out = np.empty((B, C, HW), np.float32)
    for core in range(NCORES):
        b, qc = divmod(core, NCORES // B)
        y = np.asarray(results[core]["y"]).reshape(C, QN)
        out[b, :, qc * QN:(qc + 1) * QN] = y
    return out.reshape(B, C, H, W)


def kernel(x, gamma, beta, wq, bq, wk, bk, wv, bv, wp, bp):
    nc = _get_nc()
    in_maps = make_in_maps(x, gamma, beta, wq, bq, wk, bk, wv, bv, wp, bp)
    res = run_bass_kernel_spmd(nc, in_maps, list(range(NCORES)))
    return assemble_output(res.results)


# revision 36
# speedup vs baseline: 1.1638x; 1.1638x over previous
"""Trainium2 Bass kernel for nn_AttentionBlock (GroupNorm + single-head spatial
self-attention + residual) on 8 NeuronCores.

Sharding: data-parallel over batch (2) x sequence-parallel over the query
dimension (4 chunks of 1024 of the 4096 spatial tokens). Each core gets the
full image of its batch element, ROTATED so its query chunk sits at token 0
(GroupNorm stats, key/value sets and softmax sums are permutation-invariant
over tokens, so rotation lets all 8 cores run the identical SPMD program).

v2: fp8(e4m3) DoubleRow matmuls everywhere except the final wp projection.
  - x streams to SBUF ONCE (f32r resident); GroupNorm stats (bn_stats) and
    warm-up matmuls (keeps the PE HAM un-throttled through the DMA-bound
    stats phase) consume it chunk-wise as it lands.
  - the GroupNorm scale a folds into the fp8 x copy (xn8 = a*x, quantize);
    the shift b becomes per-output-channel constants via tiny fp8 matmuls:
    q keeps qb, k's bias is DROPPED (a per-query additive logit constant is
    softmax-invariant), v's bias rides through softmax into yb.
  - projections: weights pre-quantized to fp8 on the host in DoubleRow pair
    layout [pair][128, 2, 512]; k/q use weight-stationary DoubleRow matmuls
    (contraction 256/step at 0.5 cyc/row), v uses x-stationary to produce
    vT [token, channel] directly (no transposes anywhere).
  - attention: scores^T per 128-key tile = 2 DoubleRow MMs; exp on ScalarE
    (shifted by -2.0 to keep exp < 448 = fp8 max) straight to fp8 pairs;
    row-sums via a DoubleRow ones-matmul; AV accumulates over 16 key-pairs.
  - softmax normalization deferred: 1/r via exp(-ln r) on ScalarE, broadcast
    with a K=1 matmul, folded into the PSUM->SBUF move; y = wp @ hattn (f32r)
    + yb + x (from the resident x) in one fused DVE op per tile.
"""

import sys
from collections import deque
from contextlib import ExitStack

if "/opt/trn_rl_repo" not in sys.path:
    sys.path.insert(0, "/opt/trn_rl_repo")

import ml_dtypes
import numpy as np

import concourse.bass as bass  # noqa: F401  (import keeps bass registered)
import concourse.tile as tile
from concourse import bacc, mybir
from concourse.alu_op_type import AluOpType
from concourse.bass_utils import run_bass_kernel_spmd

F32 = mybir.dt.float32
F32R = mybir.dt.float32r
F8 = mybir.dt.float8e4
AF = mybir.ActivationFunctionType
OP = AluOpType
DR = mybir.MatmulPerfMode.DoubleRow

B, C, H, W = 2, 512, 64, 64
HW = H * W          # 4096 spatial tokens
P = 128             # partitions
CT = C // P         # 4 channel tiles
CP = CT // 2        # 2 channel-tile pairs (DoubleRow contraction groups)
NCORES = 8
QN = HW // 4        # 1024 queries per core
CHW = 512           # token chunk width
NCH = HW // CHW     # 8 chunks
JT = HW // P        # 32 key tiles
JP = JT // 2        # 16 key-tile pairs
EPS = 1e-6
SCALE = float(C) ** -0.5
SHIFT = 2.0         # exp(logit - SHIFT): keeps max exp ~190 < 448 (fp8 max)
GPT = P // 16       # 8 groups per channel tile


def _build_body(nc, tc, ctx, d):
    xb_d = d["xb"]
    y_d = d["y"]

    cpool = ctx.enter_context(tc.tile_pool(name="const", bufs=1))
    ppool = ctx.enter_context(tc.tile_pool(name="persist", bufs=1))
    spool = ctx.enter_context(tc.tile_pool(name="stream", bufs=2))
    smpool = ctx.enter_context(tc.tile_pool(name="small", bufs=1))
    qpool = ctx.enter_context(tc.tile_pool(name="psum", bufs=3, space="PSUM"))

    dma_engines = [nc.gpsimd, nc.scalar, nc.sync]

    # ---- phase 1: stream x to SBUF once; GroupNorm stats per chunk ----
    ind = cpool.tile([P, GPT], F32, tag="ind")
    nc.scalar.dma_start(ind[:], d["ind"][:])
    gps = qpool.tile([GPT, 2 * CT], F32, tag="pa")
    sts = [smpool.tile([P, NCH, 6], F32, tag="st", bufs=CT, name=f"st{t}")
           for t in range(CT)]
    xres = [ppool.tile([P, HW], F32R, tag=f"xr{t}", name=f"xres{t}")
            for t in range(CT)]
    # tile-major streaming: tile t completes before t+1, so the per-tile
    # stats chains below overlap the remaining tiles' DMA
    for t in range(CT):
        for half in range(NCH // 2):
            eng = dma_engines[(t * (NCH // 2) + half) % 3]
            eng.dma_start(xres[t][:, half * 2 * CHW:(half + 1) * 2 * CHW],
                          xb_d[half, t])
    # xn8 = fp8 cast of RAW x, built chunk-wise during the DMA-bound stats
    # phase (the GroupNorm scale a folds into the fp8 weights instead)
    xn8 = [ppool.tile([P, 2, HW], F8, tag=f"xn8_{p}", name=f"xn8_{p}")
           for p in range(CP)]
    trash = qpool.tile([P, CHW], F32, tag="pr", bufs=1, name="trash")
    for t in range(CT):
        for ch in range(NCH):
            sl = slice(ch * CHW, (ch + 1) * CHW)
            nc.vector.bn_stats(sts[t][:, ch, :], xres[t][:, sl])
            if ch % 2 == 0:
                nc.scalar.copy(xn8[t // 2][:, t % 2, sl], xres[t][:, sl])
            else:
                nc.vector.tensor_copy(xn8[t // 2][:, t % 2, sl],
                                      xres[t][:, sl])
            # dummy matmuls on the freshly-landed chunks keep the PE HAM warm
            # through the DMA-bound stats phase: one ~3.4us burst on chunk 0
            # trips the SHORT window to K=8/8, then one matmul every other
            # chunk defeats the MID-idle re-throttle; results are discarded
            for _ in range(8 if (t, ch) == (0, 0) else (ch + 1) % 2):
                nc.tensor.matmul(trash[:], xres[t][:, ch * CHW:ch * CHW + P],
                                 xres[t][:, sl], start=True, stop=True)
        # per-tile stats chain overlaps the remaining tiles' DMA
        mv = smpool.tile([P, 2], F32, tag="mv", bufs=CT, name=f"mv{t}")
        nc.vector.bn_aggr(mv[:], sts[t][:])
        sq = smpool.tile([P, 1], F32, tag="sq", bufs=CT, name=f"sq{t}")
        nc.vector.tensor_tensor(sq[:], mv[:, 0:1], mv[:, 0:1], op=OP.mult)
        s2 = smpool.tile([P, 2], F32, tag="s2", bufs=CT, name=f"s2{t}")
        nc.vector.tensor_copy(s2[:, 0:1], mv[:, 0:1])
        nc.vector.tensor_tensor(s2[:, 1:2], sq[:], mv[:, 1:2], op=OP.add)
        nc.tensor.matmul(gps[:, 2 * t:2 * t + 2], ind[:], s2[:],
                         start=True, stop=True)

    # ---- small constants (after the x stream in every trigger queue) ----
    chv = []
    for t in range(CT):
        v = cpool.tile([P, 6], F32, tag=f"chv{t}", name=f"chv{t}")
        nc.gpsimd.dma_start(v[:], d["chv"][t])
        chv.append(v)
    # chv columns: gamma, beta, bq, bk, bv, bp
    indT = cpool.tile([GPT, P], F32, tag="indT")
    nc.gpsimd.dma_start(indT[:], d["indT"][:])
    # f32r projection weights (transposed [c, o]): consumed by the tiny bias
    # contracts and by the one-time a-scaled fp8 quantization below
    wts = {}
    for wi, name in enumerate(("wkT", "wvT", "wqT")):
        wts[name] = []
        for t in range(CT):
            w = cpool.tile([P, C], F32R, tag=f"{name}{t}", name=f"{name}{t}")
            dma_engines[(wi * CT + t) % 3].dma_start(w[:], d[name][t])
            wts[name].append(w)
    # fp8 ones for the DoubleRow row-sum matmul: M=16 columns because the
    # dual-fp8 LDWEIGHTS ISA check requires the pair step to be 16-aligned
    # (and fp8 memset is not a legal ISA instruction -> host constant)
    ones2 = cpool.tile([P, 2, 16], F8, tag="ones2")
    nc.gpsimd.dma_start(ones2[:], d["ones8"][:])
    ones_r32 = smpool.tile([1, P], F32R, tag="onesr32")
    nc.gpsimd.dma_start(ones_r32[:], d["onesr"][:])
    epst = smpool.tile([GPT, 1], F32, tag="eps")
    nc.vector.memset(epst[:], EPS)
    shiftt = smpool.tile([P, 1], F32, tag="shift")
    nc.vector.memset(shiftt[:], -SHIFT)

    gst = smpool.tile([GPT, 2 * CT], F32, tag="gst")
    nc.vector.tensor_copy(gst[:], gps[:])
    g3 = gst.rearrange("p (t two) -> p t two", two=2)
    msq = smpool.tile([GPT, CT], F32, tag="msq")
    nc.vector.tensor_tensor(msq[:], g3[:, :, 0], g3[:, :, 0], op=OP.mult)
    varg = smpool.tile([GPT, CT], F32, tag="varg")
    nc.vector.tensor_tensor(varg[:], g3[:, :, 1], msq[:], op=OP.subtract)
    stdg = smpool.tile([GPT, CT], F32, tag="stdg")
    nc.scalar.activation(stdg[:], varg[:], AF.Sqrt, bias=epst[:])
    # interleave (mu_t, rstd_t) columns and broadcast all groups->channels
    # with a single [K=8, M=128, N=8] indicator matmul
    mr = smpool.tile([GPT, 2 * CT], F32, tag="mr")
    mr3 = mr.rearrange("p (t two) -> p t two", two=2)
    nc.vector.tensor_copy(mr3[:, :, 0], g3[:, :, 0])
    nc.vector.reciprocal(mr3[:, :, 1], stdg[:])
    cba = qpool.tile([P, 2 * CT], F32, tag="pa")
    nc.tensor.matmul(cba[:], indT[:], mr[:], start=True, stop=True)
    cb = smpool.tile([P, 2 * CT], F32, tag="cb")
    nc.vector.tensor_copy(cb[:], cba[:])

    # per-channel Scale a (col 0) / Bias b (col 1); bvec = f32r copy of b
    sbts, bvec = [], []
    for t in range(CT):
        sbt = ppool.tile([P, 2], F32, tag=f"sb{t}")
        nc.vector.tensor_tensor(sbt[:, 0:1], cb[:, 2 * t + 1:2 * t + 2],
                                chv[t][:, 0:1], op=OP.mult)
        tmpb = smpool.tile([P, 1], F32, tag="tmpb", bufs=1)
        nc.vector.tensor_tensor(tmpb[:], cb[:, 2 * t:2 * t + 1], sbt[:, 0:1],
                                op=OP.mult)
        nc.vector.tensor_tensor(sbt[:, 1:2], chv[t][:, 1:2], tmpb[:],
                                op=OP.subtract)
        bv_ = ppool.tile([P, 2], F32R, tag=f"bvec{t}", name=f"bvec{t}")
        nc.vector.tensor_copy(bv_[:, 0:1], sbt[:, 1:2])
        nc.vector.tensor_copy(bv_[:, 1:2], sbt[:, 1:2])
        sbts.append(sbt)
        bvec.append(bv_)

    # ---- one-time a-scaled fp8 weight quantization (single rounding) ----
    # w8s[name][p][cp, t, o] = fp8( wT[(2p+t)*128+cp, o] * a[(2p+t)*128+cp] )
    w8s = {}
    for name in ("wkT", "wvT", "wqT"):
        w8s[name] = [cpool.tile([P, 2, C], F8, tag=f"{name}8_{p}",
                                name=f"{name}8_{p}") for p in range(CP)]
        for t in range(CT):
            dst = w8s[name][t // 2][:, t % 2, :]
            if t % 2 == 0:
                nc.scalar.activation(dst, wts[name][t][:], AF.Copy,
                                     scale=sbts[t][:, 0:1])
            else:
                nc.vector.tensor_scalar_mul(dst, wts[name][t][:],
                                            sbts[t][:, 0:1])

    # ---- bias-term constants from UNSCALED weights (tiny N=2 matmuls) ----
    #   qb[o] = sum_c wq[o,c] b[c] + bq    (per-partition add at the q copy)
    #   vbt[c] = sum_cin wv[c,cin] b[cin] + bv   (rides softmax into yb)
    #   yb[o] = sum_c wp[o,c] vbt[c] + bp        (y epilogue constant)
    #   (k needs NO bias: a per-query logit constant is softmax-invariant)
    def bias_contract(wtiles, rhs_tiles, outdt, addcol, tagp, two_col=False):
        outs = []
        for ot in range(CT):
            pb = qpool.tile([P, 2], F32, tag="pa")
            for t in range(CT):
                nc.tensor.matmul(pb[:], wtiles[t][:, ot * P:(ot + 1) * P],
                                 rhs_tiles[t][:, 0:2], start=(t == 0),
                                 stop=(t == CT - 1))
            w = 2 if two_col else 1
            ob = ppool.tile([P, w], outdt, tag=f"{tagp}{ot}", name=f"{tagp}{ot}")
            nc.vector.tensor_scalar(ob[:], pb[:, 0:w],
                                    chv[ot][:, addcol:addcol + 1],
                                    None, OP.add)
            outs.append(ob)
        return outs

    vbt = bias_contract(wts["wvT"], bvec, F32R, 4, "vbt", two_col=True)
    qb = bias_contract(wts["wqT"], bvec, F32, 2, "qb")

    # ---- persistent attention operands (all fp8, DoubleRow layouts) ----
    # k2[p]  : [128, j-tile, pair-slot, 128]   stationary slices [:, j, :, :]
    # q2[p]  : [128, pair-slot, 1024]          moving slices [:, :, i-half]
    # xn8[p] : [128, pair-slot, 4096]          moving (k/q) + stationary (v)
    # vT2[jp]: [128, c-tile, pair-slot, 128]   stationary slices [:, t, :, :]
    k2 = [ppool.tile([P, JT, 2, P], F8, tag=f"k2_{p}", name=f"k2_{p}")
          for p in range(CP)]
    q2 = [ppool.tile([P, 2, QN], F8, tag=f"q2_{p}", name=f"q2_{p}")
          for p in range(CP)]
    vT2 = [ppool.tile([P, CT, 2, P], F8, tag=f"vT2_{jp}", name=f"vT2_{jp}")
           for jp in range(JP)]

    # ---- phase 2: q/k/v fp8 projections, streamed over x chunks ----
    for ch in range(NCH):
        sl = slice(ch * CHW, (ch + 1) * CHW)
        for ot in range(CT):
            pk = qpool.tile([P, CHW], F32, tag="pa")
            for p in range(CP):
                nc.tensor.matmul(pk[:],
                                 w8s["wkT"][p][:, :, ot * P:(ot + 1) * P],
                                 xn8[p][:, :, sl], start=(p == 0),
                                 stop=(p == CP - 1), perf_mode=DR)
            # k write: [128, 4 j-tiles, 1, 128] strided into the pair layout
            nc.vector.tensor_copy(k2[ot // 2][:, 4 * ch:4 * ch + 4, ot % 2, :],
                                  pk[:])
        for tg in range(CT):
            jt = ch * CT + tg
            pv = qpool.tile([P, CHW], F32, tag="pa")
            for p in range(CP):
                nc.tensor.matmul(
                    pv[:], xn8[p][:, :, jt * P:(jt + 1) * P],
                    w8s["wvT"][p][:], start=(p == 0), stop=(p == CP - 1),
                    perf_mode=DR)
            dst = vT2[jt // 2][:, :, jt % 2, :]
            if tg % 2 == 1:
                nc.scalar.copy(dst, pv[:])
            else:
                nc.vector.tensor_copy(dst, pv[:])
            del dst
        if ch * CHW < QN:
            for ot in range(CT):
                pq = qpool.tile([P, CHW], F32, tag="pa")
                for p in range(CP):
                    nc.tensor.matmul(
                        pq[:], w8s["wqT"][p][:, :, ot * P:(ot + 1) * P],
                        xn8[p][:, :, sl], start=(p == 0), stop=(p == CP - 1),
                        perf_mode=DR)
                if ot % 2 == 0:
                    nc.scalar.add(q2[ot // 2][:, ot % 2, sl], pq[:],
                                  qb[ot][:, 0:1])
                else:
                    nc.vector.tensor_scalar(q2[ot // 2][:, ot % 2, sl], pq[:],
                                            qb[ot][:], None, OP.add)

    # ---- phase 3: attention, per query half ----
    # wpT (f32r) loads late: only the y epilogue needs it
    wpT = []
    for t in range(CT):
        w = cpool.tile([P, C], F32R, tag=f"wpT{t}", name=f"wpT{t}")
        nc.sync.dma_start(w[:], d["wpT"][t])
        wpT.append(w)
    yb = []
    for ot in range(CT):
        pb = qpool.tile([P, 2], F32, tag="pa")
        for t in range(CT):
            nc.tensor.matmul(pb[:], wpT[t][:, ot * P:(ot + 1) * P],
                             vbt[t][:, 0:2], start=(t == 0), stop=(t == CT - 1))
        ob = ppool.tile([P, 1], F32, tag=f"yb{ot}", name=f"yb{ot}")
        nc.vector.tensor_scalar(ob[:], pb[:, 0:1], chv[ot][:, 5:6], None, OP.add)
        yb.append(ob)

    def mk_pr():
        return qpool.tile([16, CHW], F32, tag="pr", bufs=1, name="pr")

    def mk_po():
        return [qpool.tile([P, CHW], F32, tag=f"po{t}", name=f"po{t}", bufs=1)
                for t in range(CT)]

    def score_pair(ih, jp):
        """scores^T + exp for key tiles (2jp, 2jp+1) -> one fp8 pT2 pair."""
        isl = slice(ih * CHW, (ih + 1) * CHW)
        pT2 = spool.tile([P, 2, CHW], F8, tag="pT2", bufs=6, name="pT2")
        for jj in range(2):
            j = 2 * jp + jj
            ps_ = qpool.tile([P, CHW], F32, tag="pa", name="ps")
            for p in range(CP):
                nc.tensor.matmul(ps_[:], k2[p][:, j, :, :], q2[p][:, :, isl],
                                 start=(p == 0), stop=(p == CP - 1),
                                 perf_mode=DR)
            nc.scalar.activation(pT2[:, jj, :], ps_[:], AF.Exp,
                                 scale=SCALE, bias=shiftt[:])
        return pT2

    def av_pair(pr, po, jp, pT2):
        nc.tensor.matmul(pr[:], ones2[:], pT2[:], start=(jp == 0),
                         stop=(jp == JP - 1), perf_mode=DR)
        for t in range(CT):
            nc.tensor.matmul(po[t][:], vT2[jp][:, t, :, :], pT2[:],
                             start=(jp == 0), stop=(jp == JP - 1),
                             perf_mode=DR)

    def tail_and_y(pr, po, ih, nsub=1):
        # nsub>1 splits the epilogue into query sub-slices so the final
        # drain pipelines DVE normalize / PE matmul / DMA out
        sw = CHW // nsub
        for sub in range(nsub):
            lo = ih * CHW + sub * sw
            isl = slice(lo, lo + sw)
            psl = slice(sub * sw, (sub + 1) * sw)
            rsb = spool.tile([1, sw], F32R, tag="sx", bufs=3)
            nc.vector.tensor_copy(rsb[:], pr[0:1, psl])
            # 1/r via exp(-ln(r)) on ScalarE, in place: faster than DVE's
            # iterative reciprocal and only one stream-pool slot
            nc.scalar.activation(rsb[:], rsb[:], AF.Ln)
            nc.scalar.activation(rsb[:], rsb[:], AF.Exp, scale=-1.0)
            prb = qpool.tile([P, sw], F32, tag="pa")
            nc.tensor.matmul(prb[:], ones_r32[:], rsb[:], start=True, stop=True)
            rb = spool.tile([P, sw], F32, tag="sx", bufs=3)
            nc.vector.tensor_copy(rb[:], prb[:])
            has = []
            for t in range(CT):
                ha = spool.tile([P, sw], F32R, tag=f"hx{t}", bufs=2)
                nc.vector.tensor_tensor(ha[:], po[t][:, psl], rb[:], op=OP.mult)
                has.append(ha)
            for ot in range(CT):
                py = qpool.tile([P, sw], F32, tag="pa")
                for t in range(CT):
                    nc.tensor.matmul(py[:], wpT[t][:, ot * P:(ot + 1) * P],
                                     has[t][:], start=(t == 0),
                                     stop=(t == CT - 1))
                yt = spool.tile([P, sw], F32, tag="yt", bufs=4, name="yt")
                nc.vector.scalar_tensor_tensor(yt[:], py[:], yb[ot][:, 0:1],
                                               xres[ot][:, isl],
                                               op0=OP.add, op1=OP.add)
                dma_engines[(ot + sub) % 3].dma_start(y_d[ot, :, isl], yt[:])

    # software pipeline: pair jp+1's score matmuls are emitted BEFORE pair
    # jp's rowsum/AV so the PE streams through exp's ~700ns latency instead
    # of stalling on it; KPRE extra half-1 pairs cover half-0's epilogue
    KPRE = 4
    sq_ = deque()
    pr0 = mk_pr()
    po0 = mk_po()
    sq_.append(score_pair(0, 0))
    for jp in range(JP):
        if jp + 1 < JP:
            sq_.append(score_pair(0, jp + 1))
        else:
            sq_.append(score_pair(1, 0))
        av_pair(pr0, po0, jp, sq_.popleft())
    pr1 = mk_pr()
    for jp in range(1, KPRE):
        sq_.append(score_pair(1, jp))
    tail_and_y(pr0, po0, 0)
    po1 = mk_po()
    for jp in range(JP):
        if jp + KPRE < JP:
            sq_.append(score_pair(1, jp + KPRE))
        av_pair(pr1, po1, jp, sq_.popleft())
    tail_and_y(pr1, po1, 1, nsub=2)


def build_module():
    nc = bacc.Bacc("TRN2", target_bir_lowering=False, debug=False,
                   num_devices=NCORES)
    d = {
        "xb": nc.dram_tensor("xb", [NCH // 2, CT, P, 2 * CHW], F32R,
                             kind="ExternalInput").ap(),
        "wqT": nc.dram_tensor("wqT", [CT, P, C], F32R,
                              kind="ExternalInput").ap(),
        "wkT": nc.dram_tensor("wkT", [CT, P, C], F32R,
                              kind="ExternalInput").ap(),
        "wvT": nc.dram_tensor("wvT", [CT, P, C], F32R,
                              kind="ExternalInput").ap(),
        "wpT": nc.dram_tensor("wpT", [CT, P, C], F32R,
                              kind="ExternalInput").ap(),
        "chv": nc.dram_tensor("chv", [CT, P, 6], F32, kind="ExternalInput").ap(),
        "ones8": nc.dram_tensor("ones8", [P, 2, 16], F8,
                                kind="ExternalInput").ap(),
        "onesr": nc.dram_tensor("onesr", [1, P], F32R,
                                kind="ExternalInput").ap(),
        "ind": nc.dram_tensor("ind", [P, GPT], F32, kind="ExternalInput").ap(),
        "indT": nc.dram_tensor("indT", [GPT, P], F32, kind="ExternalInput").ap(),
        "y": nc.dram_tensor("y", [CT, P, QN], F32, kind="ExternalOutput").ap(),
    }
    with tile.TileContext(nc) as tc, ExitStack() as ctx:
        _build_body(nc, tc, ctx, d)
    nc.compile()
    return nc


_CACHE = {}


def _get_nc():
    if "nc" not in _CACHE:
        _CACHE["nc"] = build_module()
    return _CACHE["nc"]


def _shared_inputs(gamma, beta, wq, bq, wk, bk, wv, bv, wp, bp):
    def wT(w):
        return np.ascontiguousarray(np.asarray(w, np.float32).T).reshape(CT, P, C)

    ind = np.zeros((P, GPT), np.float32)
    for i in range(P):
        ind[i, i // 16] = 1.0 / 16.0
    indT = np.zeros((GPT, P), np.float32)
    for i in range(P):
        indT[i // 16, i] = 1.0
    chv = np.stack([np.asarray(a, np.float32)
                    for a in (gamma, beta, bq, bk, bv, bp)],
                   axis=1).reshape(CT, P, 6)
    return {
        "wqT": wT(wq), "wkT": wT(wk), "wvT": wT(wv),
        "wpT": wT(wp),
        "chv": np.ascontiguousarray(chv),
        "ones8": np.ones((P, 2, 16), np.float32).astype(ml_dtypes.float8_e4m3fn),
        "onesr": np.ones((1, P), np.float32),
        "ind": ind, "indT": indT,
    }


def make_in_maps(x, gamma, beta, wq, bq, wk, bk, wv, bv, wp, bp):
    shared = _shared_inputs(gamma, beta, wq, bq, wk, bk, wv, bv, wp, bp)
    xf = np.asarray(x, np.float32).reshape(B, C, HW)
    in_maps = []
    for core in range(NCORES):
        b, qc = divmod(core, NCORES // B)
        xb = np.roll(xf[b], -qc * QN, axis=1)          # [C, HW]
        xt = xb.reshape(CT, P, NCH // 2, 2 * CHW).transpose(2, 0, 1, 3)
        m = dict(shared)
        m["xb"] = np.ascontiguousarray(xt)
        in_maps.append(m)
    return in_maps


def assemble_output(results):
    out = np.empty((B, C, HW), np.float32)
    for core in range(NCORES):
        b, qc = divmod(core, NCORES // B)
        y = np.asarray(results[core]["y"]).reshape(C, QN)
        out[b, :, qc * QN:(qc + 1) * QN] = y
    return out.reshape(B, C, H, W)


def kernel(x, gamma, beta, wq, bq, wk, bk, wv, bv, wp, bp):
    nc = _get_nc()
    in_maps = make_in_maps(x, gamma, beta, wq, bq, wk, bk, wv, bv, wp, bp)
    res = run_bass_kernel_spmd(nc, in_maps, list(range(NCORES)))
    return assemble_output(res.results)
